# revision 8
# baseline (speedup 1.0000x reference)
"""Trainium2 Bass kernel for nn_ComplexityAttention (GQA attention block).

Computation (B=1, S=2048, HID=2048, 16 Q heads / 4 KV heads, D=128):
  q/k/v = x @ W^T + mu @ Wm^T           (fused mu-guided projections)
  per-head RMSNorm on q, k; RoPE; causal GQA attention; out @ wo^T.

Sharding: tensor-parallel over heads across 8 NeuronCores. Core c owns
Q heads {2c, 2c+1} and KV head c//2 (KV work duplicated per core pair).
Each core produces a partial output (its heads' slice of wo applied),
host sums the 8 partials.

Device-side strategy (v3, fully software-pipelined):
  - Host pre-tiles every DRAM tensor partition-major so each logical
    load/store is ONE large DMA with 4-16KB contiguous runs per partition
    (the shared HWDGE descriptor-gen queue costs ~630ns per DMA
    instruction, so DMA COUNT is minimized).
  - The PE instruction stream is explicitly scheduled (engines execute
    in issue order): attention chunks are WOVEN into the projection
    matmul stream of later passes so the PE never waits on the scalar
    engine's exp; QK->PV runs with a lag of 2 score tiles.
  - Projections in [s, d] tiles, i-major matmul order for early PSUM
    release; RMSNorm+RoPE via fused scalar_tensor_tensor; Q/K moved to
    [d, s] with DMA transposes (no PE/PSUM involvement).
  - Softmax without max-subtraction (scores bounded by sqrt(128) after
    RMSNorm); causal masking via one static multiplicative mask on the
    diagonal tiles; triangular-N QK matmuls skip fully-masked columns.
  - Softmax denominator off the PE: DVE bf16 adds of the exp tiles +
    one gpsimd partition_all_reduce per (qc, head).
  - PV: out^T[d, q] = V^T @ expS^T accumulated over kv chunks.
  - Output projection staged in SBUF bf16, one DMA per 512-q chunk via
    the gpsimd SWDGE queue (keeps the SP queue free for input streams).

All matmul inputs are bf16 (fp32 PSUM accumulation); statistics in fp32.
"""

import sys

for _p in ("/opt/trn_rl_repo", "/root/.axon_site/_ro/trn_rl_repo"):
    if _p not in sys.path:
        sys.path.insert(0, _p)

import numpy as np
import ml_dtypes

import concourse.bass as bass
import concourse.bacc as bacc
import concourse.mybir as mybir
import concourse.bass_isa as bass_isa
import concourse.tile as tile
from concourse.bass_utils import run_bass_kernel_spmd

# Problem constants (hardcoded per contract)
B, S, HID = 1, 2048, 2048
NUM_HEADS, NUM_KV_HEADS, HEAD_DIM = 16, 4, 128
ROPE_THETA = 10000.0
EPS = 1e-6
N_CORES = 8

P = 128
KC = HID // P            # 16 contraction chunks
SC = S // P              # 16 sequence chunks of 128
QCH = 512                # attention q-chunk (one PSUM bank)
NQC = S // QCH           # 4
NPASS = 8                # projection passes (2 s-chunks each)
SCP = SC // NPASS        # s-chunks per pass = 2
OC = HID // P            # 16 output-row chunks
QK_SCALE = 1.0 / float(np.sqrt(HEAD_DIM))
LAG = 2                  # QK->PV score-tile lag (PSUM: s pool bufs)

BF16 = mybir.dt.bfloat16
F32 = mybir.dt.float32
NP_BF16 = ml_dtypes.bfloat16

_PROGRAM = {}


def _weave(a, b):
    """Issue stream b's items evenly spread among stream a's items."""
    na, nb = len(a), len(b)
    if nb == 0:
        for f in a:
            f()
        return
    if na == 0:
        for f in b:
            f()
        return
    bi = 0
    for ai, f in enumerate(a):
        f()
        while bi < nb and (bi + 1) * na <= (ai + 1) * nb:
            b[bi]()
            bi += 1
    while bi < nb:
        b[bi]()
        bi += 1


def _build_program():
    """Build the per-core Bass/Tile program (identical on all 8 cores)."""
    AF = mybir.ActivationFunctionType
    OP = mybir.AluOpType

    nc = bacc.Bacc(trn_type="TRN2", debug=False)

    # ---- DRAM I/O (all pre-tiled partition-major by the host) ----
    x_t = nc.dram_tensor("x_t", [NPASS, P, SCP * KC * P], BF16, kind="ExternalInput")
    mu_t = nc.dram_tensor("mu_t", [NPASS, P, SCP * KC * P], BF16, kind="ExternalInput")
    w_t = nc.dram_tensor("w_t", [P, KC * 512], BF16, kind="ExternalInput")
    wm_t = nc.dram_tensor("wm_t", [P, KC * 512], BF16, kind="ExternalInput")
    wo_t = nc.dram_tensor("wo_t", [P, 2 * HID], BF16, kind="ExternalInput")
    rope_t = nc.dram_tensor("rope_t", [P, 4 * SC * HEAD_DIM], BF16, kind="ExternalInput")
    out_t = nc.dram_tensor("out", [NQC, P, OC * QCH], BF16, kind="ExternalOutput")

    with tile.TileContext(nc) as tc:
        with (
            tc.tile_pool(name="persist", bufs=1) as persist,
            tc.tile_pool(name="stream", bufs=3) as stream,
            tc.tile_pool(name="tmp", bufs=4) as tmp,
            tc.tile_pool(name="small", bufs=6) as small,
            tc.tile_pool(name="expp", bufs=6) as expp,
            tc.tile_pool(name="dena", bufs=2) as dena,
            tc.tile_pool(name="ostage", bufs=2) as ostage,
            tc.tile_pool(name="ps_proj", bufs=2, space="PSUM") as ps_proj,
            tc.tile_pool(name="ps_s", bufs=LAG, space="PSUM") as ps_s,
            tc.tile_pool(name="ps_out", bufs=2, space="PSUM") as ps_out,
            tc.tile_pool(name="ps_o", bufs=2, space="PSUM") as ps_o,
        ):
            # ---- persistent SBUF tensors ----
            w_sb = persist.tile([P, KC, 512], BF16, name="w_sb")
            wm_sb = persist.tile([P, KC, 512], BF16, name="wm_sb")
            wo_sb = persist.tile([P, 2, HID], BF16, name="wo_sb")
            rope_sb = persist.tile([P, 4, SC, HEAD_DIM], BF16, name="rope_sb")
            qt_sb = [persist.tile([P, S], BF16, name=f"qt{h}_sb") for h in range(2)]
            kt_sb = persist.tile([P, S], BF16, name="kt_sb")
            v_sb = persist.tile([P, SC, HEAD_DIM], BF16, name="v_sb")
            attn_sb = [persist.tile([P, S], BF16, name=f"attn{c}_sb") for c in range(2)]
            eps_sb = persist.tile([P, 1], F32, name="eps_sb")
            mask = persist.tile([P, P], BF16, name="mask")

            nc.gpsimd.memset(eps_sb[:], EPS)
            # keep 1.0 where (q_local - kv_local) >= 0, else 0
            nc.gpsimd.memset(mask[:], 1.0)
            nc.gpsimd.affine_select(
                out=mask[:],
                in_=mask[:],
                compare_op=mybir.AluOpType.is_ge,
                fill=0.0,
                base=0,
                pattern=[[1, P]],
                channel_multiplier=-1,
            )

            # (head slice in packed 512-wide projection, cos table, sin table,
            #  destination transposed buffer)
            norm_specs = [
                (2, 2, 3, kt_sb),
                (0, 0, 1, qt_sb[0]),
                (1, 0, 1, qt_sb[1]),
            ]

            xt_tiles = {}
            mt_tiles = {}

            def load_pass(p):
                xt_tiles[p] = stream.tile([P, SCP, KC, P], BF16, tag="xt", name="xt")
                nc.sync.dma_start(xt_tiles[p][:], x_t.ap()[p])
                mt_tiles[p] = stream.tile([P, SCP, KC, P], BF16, tag="mt", name="mt")
                nc.sync.dma_start(mt_tiles[p][:], mu_t.ap()[p])

            def norm_one(p, i, ps):
                """RMSNorm + RoPE + DMA-transpose for s-chunk i of pass p."""
                sc = p * SCP + i
                for hidx, ct, st, dst in norm_specs:
                    off = hidx * P
                    sqv = tmp.tile([P, HEAD_DIM], F32, tag="sqv", name="sqv")
                    var = small.tile([P, 1], F32, tag="var", name="var")
                    nc.scalar.activation(
                        sqv[:], ps[:, off : off + P], AF.Square, accum_out=var[:]
                    )
                    std = small.tile([P, 1], F32, tag="std", name="std")
                    nc.scalar.activation(
                        std[:], var[:], AF.Sqrt, scale=1.0 / HEAD_DIM, bias=eps_sb[:]
                    )
                    rstd = small.tile([P, 1], F32, tag="rstd", name="rstd")
                    nc.vector.reciprocal(rstd[:], std[:])
                    t1 = tmp.tile([P, HEAD_DIM], F32, tag="t1", name="t1")
                    nc.vector.scalar_tensor_tensor(
                        t1[:], ps[:, off : off + P], rstd[:],
                        rope_sb[:, ct, sc, :], op0=OP.mult, op1=OP.mult,
                    )
                    t2 = tmp.tile([P, HEAD_DIM], F32, tag="t2", name="t2")
                    nc.vector.scalar_tensor_tensor(
                        t2[:, 0:64], ps[:, off + 64 : off + P], rstd[:],
                        rope_sb[:, st, sc, 0:64], op0=OP.mult, op1=OP.mult,
                    )
                    nc.vector.scalar_tensor_tensor(
                        t2[:, 64:P], ps[:, off : off + 64], rstd[:],
                        rope_sb[:, st, sc, 64:P], op0=OP.mult, op1=OP.mult,
                    )
                    qsd = tmp.tile([P, HEAD_DIM], BF16, tag="qsd", name="qsd")
                    nc.vector.tensor_add(qsd[:], t1[:], t2[:])
                    # [s, d] -> [d, s] without touching PE or PSUM
                    nc.sync.dma_start_transpose(
                        dst[:, sc * P : (sc + 1) * P], qsd[:]
                    )
                # V: plain copy (cast) into [s, d] layout
                nc.scalar.copy(v_sb[:, sc, :], ps[:, 384:512])

            def proj_stream(p):
                """Issue items for pass p: i-major matmuls + inline norm."""
                items = []
                psums = [
                    ps_proj.tile([P, 512], F32, tag="proj", name=f"proj{p}_{i}")
                    for i in range(SCP)
                ]
                xt, mt = xt_tiles.pop(p), mt_tiles.pop(p)
                for i in range(SCP):
                    for kc in range(KC):
                        items.append(lambda kc=kc, i=i: nc.tensor.matmul(
                            psums[i][:], xt[:, i, kc, :], w_sb[:, kc, :],
                            start=(kc == 0), stop=False,
                        ))
                    for kc in range(KC):
                        items.append(lambda kc=kc, i=i: nc.tensor.matmul(
                            psums[i][:], mt[:, i, kc, :], wm_sb[:, kc, :],
                            start=False, stop=(kc == KC - 1),
                        ))
                    items.append(lambda i=i: norm_one(p, i, psums[i]))
                return items

            def attn_stream(qc):
                """QK/exp/PV items for q chunk qc with a LAG-deep score
                pipeline; returns (items, finish) where finish() issues the
                softmax division."""
                jpq = QCH // P
                J = jpq * qc + jpq          # kv chunks for this q chunk
                n = 2 * J                   # (j, h) slots
                out_ps = [
                    ps_out.tile([P, QCH], F32, tag="out", name=f"out_ps{h}")
                    for h in range(2)
                ]
                den_ac = [
                    dena.tile([P, QCH], BF16, tag="dac", name=f"dac{h}")
                    for h in range(2)
                ]
                e_tiles = {}

                def front(idx):
                    j, h = idx // 2, idx % 2
                    r = j - jpq * qc
                    s_ps = ps_s.tile([P, QCH], F32, tag="s", name="s_ps")
                    col0 = P * r if r > 0 else 0
                    nc.tensor.matmul(
                        s_ps[:, col0:],
                        kt_sb[:, j * P : (j + 1) * P],
                        qt_sb[h][:, qc * QCH + col0 : (qc + 1) * QCH],
                        start=True, stop=True,
                    )
                    e = expp.tile([P, QCH], BF16, tag="e", name="e")
                    if col0 > 0:
                        nc.vector.memset(e[:, :col0], 0.0)
                    nc.scalar.activation(
                        e[:, col0:], s_ps[:, col0:], AF.Exp, scale=QK_SCALE
                    )
                    if r >= 0:
                        nc.vector.tensor_mul(
                            e[:, P * r : P * (r + 1)],
                            e[:, P * r : P * (r + 1)],
                            mask[:],
                        )
                    e_tiles[idx] = e

                def back(idx):
                    j, h = idx // 2, idx % 2
                    e = e_tiles.pop(idx)
                    nc.tensor.matmul(
                        out_ps[h][:], v_sb[:, j, :], e[:],
                        start=(j == 0), stop=(j == J - 1),
                    )
                    if j == 0:
                        nc.vector.tensor_copy(den_ac[h][:], e[:])
                    else:
                        nc.vector.tensor_add(den_ac[h][:], den_ac[h][:], e[:])

                items = []
                for idx in range(n + LAG):
                    ops = []
                    if idx < n:
                        ops.append(lambda idx=idx: front(idx))
                    if idx >= LAG:
                        ops.append(lambda idx=idx: back(idx - LAG))
                    items.append(lambda ops=ops: [f() for f in ops])

                def finish():
                    q_sl = slice(qc * QCH, (qc + 1) * QCH)
                    for h in range(2):
                        den_f = dena.tile([P, QCH], F32, tag="dfl", name="dfl")
                        nc.gpsimd.partition_all_reduce(
                            den_f[:], den_ac[h][:], channels=P,
                            reduce_op=bass_isa.ReduceOp.add,
                        )
                        rec = small.tile([1, QCH], F32, tag="rec", name="rec")
                        nc.vector.reciprocal(rec[:], den_f[0:1, :])
                        rdb = dena.tile([P, QCH], F32, tag="rdb", name="rdb")
                        nc.gpsimd.partition_broadcast(rdb[:], rec[:])
                        nc.vector.tensor_mul(
                            attn_sb[h][:, q_sl], out_ps[h][:], rdb[:]
                        )
                return items, finish

            def wo_stream(qc, dve_copies=False):
                """Output projection items for q chunk qc."""
                q_sl = slice(qc * QCH, (qc + 1) * QCH)
                ob = ostage.tile([P, OC * QCH], BF16, tag="ob", name="ob")
                items = []

                def one(oc):
                    o_ps = ps_o.tile([P, QCH], F32, tag="o", name="o_ps")
                    for c in range(2):
                        nc.tensor.matmul(
                            o_ps[:],
                            wo_sb[:, c, oc * P : (oc + 1) * P],
                            attn_sb[c][:, q_sl],
                            start=(c == 0), stop=(c == 1),
                        )
                    dst = ob[:, oc * QCH : (oc + 1) * QCH]
                    if dve_copies or oc % 2 == 1:
                        nc.vector.tensor_copy(dst, o_ps[:])
                    else:
                        nc.scalar.copy(dst, o_ps[:])

                for oc in range(OC):
                    items.append(lambda oc=oc: one(oc))
                items.append(lambda: nc.gpsimd.dma_start(out_t.ap()[qc], ob[:]))
                return items

            # ---------------- schedule ----------------
            # prologue: input/weight streams (SP queue, ordered so the PE can
            # start on pass 0's x matmuls as early as possible)
            xt_tiles[0] = stream.tile([P, SCP, KC, P], BF16, tag="xt", name="xt")
            nc.sync.dma_start(xt_tiles[0][:], x_t.ap()[0])
            nc.sync.dma_start(w_sb[:, 0:8, :], w_t.ap()[:, 0 : 8 * 512])
            nc.sync.dma_start(w_sb[:, 8:16, :], w_t.ap()[:, 8 * 512 :])
            mt_tiles[0] = stream.tile([P, SCP, KC, P], BF16, tag="mt", name="mt")
            nc.sync.dma_start(mt_tiles[0][:], mu_t.ap()[0])
            nc.sync.dma_start(wm_sb[:, 0:8, :], wm_t.ap()[:, 0 : 8 * 512])
            nc.sync.dma_start(wm_sb[:, 8:16, :], wm_t.ap()[:, 8 * 512 :])
            load_pass(1)
            nc.scalar.dma_start(rope_sb[:], rope_t.ap()[:])
            nc.scalar.dma_start(wo_sb[:], wo_t.ap()[:])

            # steps 0..7: pass p, with attention chunk (p//2 - 1) woven in on
            # even steps >= 2 and its wo on the following odd step (the
            # division is issued right before the wo stream that reads it)
            pending_div = None
            for p in range(NPASS):
                if p + 2 < NPASS:
                    load_pass(p + 2)
                side = []
                fin = None
                if p >= 2 and p % 2 == 0:
                    side, fin = attn_stream(p // 2 - 1)
                elif p in (3, 5):
                    pending_div()
                    pending_div = None
                    side = wo_stream(p // 2 - 1)
                _weave(proj_stream(p), side)
                if fin is not None:
                    pending_div = fin
            # tail: attention chunk 3 woven with wo chunk 2, then div3 + wo3
            pending_div()  # div2
            a3, fin3 = attn_stream(3)
            _weave(a3, wo_stream(2, dve_copies=False))
            fin3()
            for f in wo_stream(3, dve_copies=False):
                f()

    nc.compile()
    return nc


def _get_program():
    if "p" not in _PROGRAM:
        _PROGRAM["p"] = _build_program()
    return _PROGRAM["p"]


def _host_prepare(inputs):
    """Shard + lay out inputs for the 8 cores (all partition-major tiled)."""
    hs = np.asarray(inputs["hidden_states"], dtype=np.float32).reshape(S, HID)
    mu = np.asarray(inputs["mu_prev"], dtype=np.float32).reshape(S, HID)
    wq = np.asarray(inputs["wq"], dtype=np.float32)
    wk = np.asarray(inputs["wk"], dtype=np.float32)
    wv = np.asarray(inputs["wv"], dtype=np.float32)
    wo = np.asarray(inputs["wo"], dtype=np.float32)
    wmq = np.asarray(inputs["wmq"], dtype=np.float32)
    wmk = np.asarray(inputs["wmk"], dtype=np.float32)
    wmv = np.asarray(inputs["wmv"], dtype=np.float32)
    qw = np.asarray(inputs["q_norm_w"], dtype=np.float32)
    kw = np.asarray(inputs["k_norm_w"], dtype=np.float32)

    def tile_xT(a):
        # [S, HID] -> [NPASS, P(hid%128), SCP*KC*P]: x_t[p, hp, (i, kc, j)] =
        # a[(p*SCP+i)*128 + j, kc*128 + hp]
        t = a.reshape(NPASS, SCP, P, KC, P)          # [p, i, j(s), kc, hp]
        t = t.transpose(0, 4, 1, 3, 2)               # [p, hp, i, kc, j]
        return np.ascontiguousarray(t).astype(NP_BF16).reshape(NPASS, P, SCP * KC * P)

    x_t = tile_xT(hs)
    mu_t = tile_xT(mu)

    # RoPE tables in [s, d] layout with rotate-half sign and norm weight baked
    # in, packed [P(s%128), 4 tables, SC, D]
    inv = 1.0 / (ROPE_THETA ** (np.arange(0, HEAD_DIM, 2, dtype=np.float32) / HEAD_DIM))
    ang = np.arange(S, dtype=np.float32)[:, None] * inv[None, :]  # [S, 64]
    emb = np.concatenate([ang, ang], axis=-1)  # [S, 128]
    cos_e = np.cos(emb)
    sin_e = np.sin(emb)
    sin_s = np.concatenate([-sin_e[:, :64], sin_e[:, 64:]], axis=-1)

    def tables(w):
        w_shift = np.concatenate([w[64:], w[:64]])
        return cos_e * w[None, :], sin_s * w_shift[None, :]

    cq, sq = tables(qw)
    ck, sk = tables(kw)
    rope = np.stack([cq, sq, ck, sk], axis=0)        # [4, S, D]
    rope = rope.reshape(4, SC, P, HEAD_DIM).transpose(2, 0, 1, 3)  # [P, 4, SC, D]
    rope_t = np.ascontiguousarray(rope).astype(NP_BF16).reshape(P, 4 * SC * HEAD_DIM)

    def tile_w(a):
        # [HID, 512] -> [P(hid%128), KC*512]
        t = a.reshape(KC, P, 512).transpose(1, 0, 2)
        return np.ascontiguousarray(t).astype(NP_BF16).reshape(P, KC * 512)

    in_maps = []
    for c in range(N_CORES):
        g = c // 2
        wq_s = wq[256 * c : 256 * (c + 1)]      # [256, HID]
        wmq_s = wmq[256 * c : 256 * (c + 1)]
        wk_s = wk[P * g : P * (g + 1)]          # [128, HID]
        wmk_s = wmk[P * g : P * (g + 1)]
        wv_s = wv[P * g : P * (g + 1)]
        wmv_s = wmv[P * g : P * (g + 1)]
        w_all = np.concatenate([wq_s.T, wk_s.T, wv_s.T], axis=1)     # [HID, 512]
        wm_all = np.concatenate([wmq_s.T, wmk_s.T, wmv_s.T], axis=1)
        # wo slice for this core's 256 head-cols: [P(in%128), 2, HID]
        woT_c = wo[:, 256 * c : 256 * (c + 1)].T.reshape(2, P, HID).transpose(1, 0, 2)
        in_maps.append(
            {
                "x_t": x_t,
                "mu_t": mu_t,
                "w_t": tile_w(w_all),
                "wm_t": tile_w(wm_all),
                "wo_t": np.ascontiguousarray(woT_c).astype(NP_BF16).reshape(P, 2 * HID),
                "rope_t": rope_t,
            }
        )
    return in_maps


def run(inputs, trace=False):
    """Run the SPMD kernel; returns (full_output, exec_time_ns_or_None)."""
    nc = _get_program()
    in_maps = _host_prepare(inputs)
    res = run_bass_kernel_spmd(
        nc, in_maps, core_ids=list(range(N_CORES)), trace=trace
    )
    total = np.zeros((HID, S), dtype=np.float32)
    for c in range(N_CORES):
        # out_t[qc, p, oc, j] = partialT[oc*128+p, qc*512+j]
        a = res.results[c]["out"].reshape(NQC, P, OC, QCH).astype(np.float32)
        total += a.transpose(2, 1, 0, 3).reshape(HID, S)
    out = np.ascontiguousarray(total.T).reshape(B, S, HID).astype(np.float32)
    return out, res.exec_time_ns


def kernel(**inputs) -> np.ndarray:
    out, _ = run(inputs, trace=False)
    return out
